# revision 9
# baseline (speedup 1.0000x reference)
"""MoE layer (top-2 routing, 8 experts) for Trainium2 across 8 NeuronCores.

Strategy (expert-parallel):
  - The gate (logits = x @ Wg, top-2 + softmax) is computed on host; it is
    ~0.03% of the layer's FLOPs. Tokens are gathered per selected expert on
    host (the "all-to-all dispatch" of the sharding hint, done at input
    sharding time), padded to a common capacity C, one expert per core.
  - Each core runs its expert's FFN on its routed tokens:
        yT = (gelu(W1.T-tiled matmuls of x) @ W2 + b2) * gate_weight
    entirely in transposed [feature, token] layout so no on-chip transposes
    are needed. Matmul inputs are bf16 (f32 PSUM accumulation).
  - Host combines: out[t] = contrib(expert0(t)) + contrib(expert1(t)) via two
    vectorized gathers (gate weights were already applied on-device).

Per-core compute: 2 * C * D * H MACs (C = max expert count, ~1091 for the
reference routing) -> ~18.3 GFLOP -> ~295 us at the measured 2.0 GHz PE
clock; measured ~298 us/iteration on HW (~99% of that roofline).
"""

import os
from contextlib import ExitStack

import ml_dtypes
import numpy as np

import concourse.bass as bass
import concourse.tile as tile
from concourse import bacc, mybir
from concourse.bass_utils import run_bass_kernel_spmd

# Under axon without the NTFF hook module, trace=True would crash on import.
try:  # pragma: no cover
    import antenv.axon_hooks  # noqa: F401
except ImportError:
    os.environ.setdefault("BASS_NEVER_TRACE", "1")

BF16 = ml_dtypes.bfloat16
D, H, O, E, TOPK = 1024, 4096, 1024, 8, 2
P = 128
N_CORES = 8
N_D, N_H, N_O = D // P, H // P, O // P  # 8, 32, 8 k/m tiles

_CACHE: dict[int, bass.Bass] = {}


def _token_tiles(C):
    """Split capacity C into near-equal moving-dim chunks <= 512.

    Equal chunks beat a [512, ..., small-tail] split: a matmul's issue rate is
    max(N/2.4GHz, LDWEIGHTS ~107ns), so chunks below ~256 are LDW-bound and
    waste PE cycles. E.g. C=1152: 3x384 = ~246us vs [512,512,128] = ~273us.
    """
    n_chunks = -(-C // 512)
    base, rem = divmod(C, n_chunks)
    tiles, t0 = [], 0
    for i in range(n_chunks):
        n = base + (1 if i < rem else 0)
        tiles.append((t0, n))
        t0 += n
    return tiles


def _dedup_ldweights(nc, enabled: bool = True) -> int:
    """Remove InstLdweights whose weights-AP matches the previous PE weight
    load and which carry no sync info — the PE weight buffer already holds
    that tile, so the reload is a pure waste of PE issue slots.

    PE executes its instructions in block order; LDW/MM are the only PE
    instructions here and matmuls never modify the weight buffer.
    """
    if not enabled:
        return 0
    removed = 0
    for blk in nc.m.functions[0].blocks:
        last_key = None
        keep = []
        for inst in blk.instructions:
            if isinstance(inst, mybir.InstLdweights):
                si = inst.sync_info
                clean = si is None or (not si.on_wait and not si.on_update)
                key = (str(inst.ins[0]), str(inst.perf_mode),
                       str(inst.tile_position))
                if clean and key == last_key:
                    removed += 1
                    continue
                last_key = key
            elif isinstance(inst, mybir.InstMatmult):
                pass
            elif not isinstance(inst, (mybir.InstDMACopy, mybir.InstActivation,
                                       mybir.InstTensorTensor)):
                # control flow / drains / barriers: be conservative
                last_key = None
            keep.append(inst)
        blk.instructions[:] = keep
    return removed


def _build(C: int, iters: int = 1, degenerate_w: bool = False) -> bass.Bass:
    """One expert's FFN over C routed tokens, [feature, token] layout.

    Inputs (per core): xt [D, C] bf16 (tokens transposed), w1 [D, H] bf16,
    w2 [H, O] bf16, b1 [H] f32, b2 [O] f32. Output:
    yt [O, C] bf16 = gelu(x@w1+b1)@w2 + b2 (gate weights applied on host
    during combine; padding rows are never gathered there, so they need no
    zeroing on device).
    """
    f32, bf16 = mybir.dt.float32, mybir.dt.bfloat16
    nc = bacc.Bacc("TRN2", target_bir_lowering=False, debug=False,
                   num_devices=N_CORES)
    xt_d = nc.dram_tensor("xt", [D, C], bf16, kind="ExternalInput").ap()
    w1_d = nc.dram_tensor("w1", [D, H], bf16, kind="ExternalInput").ap()
    w2_d = nc.dram_tensor("w2", [H, O], bf16, kind="ExternalInput").ap()
    # b1/b2 arrive pre-transposed as [128, m] with b[p, m] = bias[m*128 + p]
    # (plain contiguous DMAs — fancy strided/broadcast DMA patterns fan out
    # across queues and blow the per-instruction sync-wait limit on their
    # first consumer).
    b1_d = nc.dram_tensor("b1", [P, N_H], f32, kind="ExternalInput").ap()
    b2_d = nc.dram_tensor("b2", [P, N_O], f32, kind="ExternalInput").ap()
    yt_d = nc.dram_tensor("yt", [O, C], bf16, kind="ExternalOutput").ap()

    with tile.TileContext(nc) as tc, ExitStack() as ctx:
        wpool = ctx.enter_context(tc.tile_pool(name="weights", bufs=1))
        xpool = ctx.enter_context(tc.tile_pool(name="xin", bufs=1))
        hpool = ctx.enter_context(tc.tile_pool(name="hts", bufs=34))
        ppool1 = ctx.enter_context(tc.tile_pool(name="ps1", bufs=2, space="PSUM"))
        ppool2 = ctx.enter_context(tc.tile_pool(name="ps2", bufs=2, space="PSUM"))
        # y DMAs have ~2us completion latency; with few bufs the
        # buffer-reuse dependency chains those latencies onto the critical
        # path (ACT -> PSUM -> PE backpressure). 8 bufs keeps them pipelined.
        ypool = ctx.enter_context(tc.tile_pool(name="yout", bufs=8))

        w1_sb = wpool.tile([P, N_D, H], bf16)   # 64 KB/partition
        w2_sb = wpool.tile([P, N_H, O], bf16)   # 64 KB/partition
        b1_sb = wpool.tile([P, N_H], f32)
        b2_sb = wpool.tile([P, N_O], f32)

        # DMA emission order == HWDGE queue order == consumption order:
        # tile-0 activations and biases first (the first matmuls need them),
        # then W1 chunked as phase A walks it, then W2 (first needed ~80us in).
        token_tiles = _token_tiles(C)
        xt_tiles = {}
        if iters == 1:
            (t0_first, nt_first) = token_tiles[0]
            xt_sb = xpool.tile([P, N_D, 512], bf16, tag="xt", name="xt0")
            for d in range(N_D):
                nc.sync.dma_start(
                    out=xt_sb[:, d, :nt_first],
                    in_=xt_d[d * P:(d + 1) * P, t0_first:t0_first + nt_first])
            xt_tiles[0] = xt_sb
        for hc in range(8):
            c0, c1 = hc * 512, (hc + 1) * 512
            for d in range(N_D):
                nc.sync.dma_start(out=w1_sb[:, d, c0:c1],
                                  in_=w1_d[d * P:(d + 1) * P, c0:c1])
            if hc == 0:
                # b1 is first read at the first gelu (~13us in); b2/g later.
                nc.sync.dma_start(out=b1_sb[:], in_=b1_d[:])
                nc.sync.dma_start(out=b2_sb[:], in_=b2_d[:])
        for h in range(N_H):
            nc.sync.dma_start(out=w2_sb[:, h, :],
                              in_=w2_d[h * P:(h + 1) * P, :])

        gelu = mybir.ActivationFunctionType.Gelu
        copy = mybir.ActivationFunctionType.Identity

        loop_ctx = ExitStack()
        if iters > 1:
            # timing-only variant: repeat the whole compute body on-device so
            # (wall(iters) - wall(1)) / (iters - 1) isolates HW exec time from
            # the axon dispatch/data-shipping overhead.
            loop_ctx.enter_context(tc.For_i(0, iters, 1))
        ctx.enter_context(loop_ctx)

        for it, (t0, nt) in enumerate(token_tiles):
            if it in xt_tiles:
                xt_sb = xt_tiles[it]
            else:
                xt_sb = xpool.tile([P, N_D, 512], bf16, tag="xt",
                                   name=f"xt{it}")
                for d in range(N_D):
                    nc.sync.dma_start(out=xt_sb[:, d, :nt],
                                      in_=xt_d[d * P:(d + 1) * P, t0:t0 + nt])
            # Phase A: hT[m*128:(m+1)*128, t] = gelu(x @ w1 + b1) per h-tile
            hts = []
            for m in range(N_H):
                ps = ppool1.tile([P, 512], f32, tag="ps1")
                for d in range(N_D):
                    lw = (w1_sb[:, 0, 0:P] if degenerate_w
                          else w1_sb[:, d, m * P:(m + 1) * P])
                    nc.tensor.matmul(ps[:, :nt], lhsT=lw,
                                     rhs=xt_sb[:, d, :nt],
                                     start=(d == 0), stop=(d == N_D - 1))
                ht = hpool.tile([P, 512], bf16, tag="ht")
                nc.scalar.activation(ht[:, :nt], ps[:, :nt], gelu,
                                     bias=b1_sb[:, m:m + 1])
                hts.append(ht)
            # Phase B: yT[o*128:(o+1)*128, t] = (hT.T-contraction @ w2 + b2)*g
            for o in range(N_O):
                ps2 = ppool2.tile([P, 512], f32, tag="ps2")
                for h in range(N_H):
                    lw = (w1_sb[:, 0, 0:P] if degenerate_w
                          else w2_sb[:, h, o * P:(o + 1) * P])
                    nc.tensor.matmul(ps2[:, :nt], lhsT=lw,
                                     rhs=hts[h][:, :nt],
                                     start=(h == 0), stop=(h == N_H - 1))
                yb = ypool.tile([P, 512], bf16, tag="yb")
                nc.scalar.activation(yb[:, :nt], ps2[:, :nt], copy,
                                     bias=b2_sb[:, o:o + 1])
                nc.sync.dma_start(out=yt_d[o * P:(o + 1) * P, t0:t0 + nt],
                                  in_=yb[:, :nt])
    nc.compile()
    return nc


def _build_ilv(C: int, iters: int = 1) -> bass.Bass:
    """Token-tile-interleaved variant: per stationary weight tile, the 3
    token tiles' matmuls are emitted back-to-back, so 2 of 3 LDWEIGHTS are
    adjacent duplicates and _dedup_ldweights removes them (~37ns of PE issue
    each). Costs: all token tiles' ht must be live at once (96KB/partition
    at nt=512), so w2 is NOT resident — it streams per o-block into a 2-slot
    window (2MB per o, ~6us, hidden under ~18us of per-o matmuls).
    Token tiles are capped at 448 so ht fits (3x28KB + w1 64KB + window 14KB
    + xt 21KB < 208KB/partition budget).
    """
    f32, bf16 = mybir.dt.float32, mybir.dt.bfloat16
    nc = bacc.Bacc("TRN2", target_bir_lowering=False, debug=False,
                   num_devices=N_CORES)
    xt_d = nc.dram_tensor("xt", [D, C], bf16, kind="ExternalInput").ap()
    w1_d = nc.dram_tensor("w1", [D, H], bf16, kind="ExternalInput").ap()
    w2_d = nc.dram_tensor("w2", [H, O], bf16, kind="ExternalInput").ap()
    b1_d = nc.dram_tensor("b1", [P, N_H], f32, kind="ExternalInput").ap()
    b2_d = nc.dram_tensor("b2", [P, N_O], f32, kind="ExternalInput").ap()
    yt_d = nc.dram_tensor("yt", [O, C], bf16, kind="ExternalOutput").ap()

    token_tiles = []
    n_chunks = -(-C // 448)
    base, rem = divmod(C, n_chunks)
    t0 = 0
    for i in range(n_chunks):
        n = base + (1 if i < rem else 0)
        token_tiles.append((t0, n))
        t0 += n
    ntt = len(token_tiles)
    nt_max = max(n for _, n in token_tiles)

    with tile.TileContext(nc) as tc, ExitStack() as ctx:
        wpool = ctx.enter_context(tc.tile_pool(name="weights", bufs=1))
        xpool = ctx.enter_context(tc.tile_pool(name="xin", bufs=ntt))
        hpool = ctx.enter_context(tc.tile_pool(name="hts", bufs=ntt))
        w2pool = ctx.enter_context(tc.tile_pool(name="w2win", bufs=2))
        ppool = ctx.enter_context(tc.tile_pool(name="ps", bufs=8, space="PSUM"))
        ypool = ctx.enter_context(tc.tile_pool(name="yout", bufs=8))

        w1_sb = wpool.tile([P, N_D, H], bf16)   # 64 KB/partition
        b1_sb = wpool.tile([P, N_H], f32)
        b2_sb = wpool.tile([P, N_O], f32)

        for hc in range(8):
            c0, c1 = hc * 512, (hc + 1) * 512
            for d in range(N_D):
                nc.sync.dma_start(out=w1_sb[:, d, c0:c1],
                                  in_=w1_d[d * P:(d + 1) * P, c0:c1])
            if hc == 0:
                nc.sync.dma_start(out=b1_sb[:], in_=b1_d[:])
                nc.sync.dma_start(out=b2_sb[:], in_=b2_d[:])

        gelu = mybir.ActivationFunctionType.Gelu
        copy = mybir.ActivationFunctionType.Identity

        loop_ctx = ExitStack()
        if iters > 1:
            loop_ctx.enter_context(tc.For_i(0, iters, 1))
        ctx.enter_context(loop_ctx)

        xts, hts = [], []
        for it, (t0, nt) in enumerate(token_tiles):
            xt_sb = xpool.tile([P, N_D, nt_max], bf16, tag="xt", name=f"xt{it}")
            for d in range(N_D):
                nc.sync.dma_start(out=xt_sb[:, d, :nt],
                                  in_=xt_d[d * P:(d + 1) * P, t0:t0 + nt])
            xts.append(xt_sb)
            hts.append(hpool.tile([P, N_H, nt_max], bf16, tag="ht",
                                  name=f"ht{it}"))
        # Phase A: hT = gelu(x @ w1 + b1), all token tiles interleaved
        for m in range(N_H):
            pss = [ppool.tile([P, 512], f32, tag="ps", name=f"psA{m}_{t}")
                   for t in range(ntt)]
            for d in range(N_D):
                lw = w1_sb[:, d, m * P:(m + 1) * P]
                for t, (t0, nt) in enumerate(token_tiles):
                    nc.tensor.matmul(pss[t][:, :nt], lhsT=lw,
                                     rhs=xts[t][:, d, :nt],
                                     start=(d == 0), stop=(d == N_D - 1))
            for t, (t0, nt) in enumerate(token_tiles):
                nc.scalar.activation(hts[t][:, m, :nt], pss[t][:, :nt], gelu,
                                     bias=b1_sb[:, m:m + 1])
        # Phase B: yT = hT.T-contraction @ w2 + b2; w2 streamed per o-block
        for o in range(N_O):
            w2blk = w2pool.tile([P, N_H, P], bf16, tag="w2")
            for h in range(N_H):
                nc.sync.dma_start(out=w2blk[:, h, :],
                                  in_=w2_d[h * P:(h + 1) * P,
                                           o * P:(o + 1) * P])
            pss = [ppool.tile([P, 512], f32, tag="ps", name=f"psB{o}_{t}")
                   for t in range(ntt)]
            for h in range(N_H):
                lw = w2blk[:, h, :]
                for t, (t0, nt) in enumerate(token_tiles):
                    nc.tensor.matmul(pss[t][:, :nt], lhsT=lw,
                                     rhs=hts[t][:, h, :nt],
                                     start=(h == 0), stop=(h == N_H - 1))
            for t, (t0, nt) in enumerate(token_tiles):
                yb = ypool.tile([P, 512], bf16, tag="yb")
                nc.scalar.activation(yb[:, :nt], pss[t][:, :nt], copy,
                                     bias=b2_sb[:, o:o + 1])
                nc.sync.dma_start(out=yt_d[o * P:(o + 1) * P, t0:t0 + nt],
                                  in_=yb[:, :nt])
    removed = _dedup_ldweights(nc)
    assert removed > 0, "expected adjacent duplicate LDWEIGHTS to dedup"
    nc.compile()
    return nc


MODE = os.environ.get("KMODE", "base")


def _prepare(x, Wg, W1, b1, W2, b2):
    """Host-side gating + per-expert gather. Returns (in_maps, glob, w, C, B, S)."""
    B, S, Dx = x.shape
    assert Dx == D and Wg.shape == (D, E), (x.shape, Wg.shape)
    T = B * S
    xf = np.ascontiguousarray(x.reshape(T, D), dtype=np.float32)
    logits = xf.astype(np.float64) @ Wg.astype(np.float64)
    top_i = np.argpartition(-logits, TOPK - 1, axis=1)[:, :TOPK]  # [T, 2]
    lv = np.take_along_axis(logits, top_i, axis=1)
    lv -= lv.max(axis=1, keepdims=True)
    ex = np.exp(lv)
    w = ex / ex.sum(axis=1, keepdims=True)  # [T, 2] softmax over the pair

    flat_e = top_i.reshape(-1)      # pair p = 2*t + k -> expert id
    flat_w = w.reshape(-1)
    counts = np.bincount(flat_e, minlength=E)
    # Tokens are the matmul free dim, so capacity needs no alignment at all;
    # every extra padded token costs PE time on all 8 cores.
    C = max(1024, int(counts.max()))

    xt_bf = np.ascontiguousarray(xf.T).astype(BF16)  # [D, T]
    W1b = W1.astype(BF16)
    W2b = W2.astype(BF16)

    in_maps = []
    glob = np.empty(2 * T, dtype=np.int64)  # pair -> row in stacked outputs
    for e in range(E):
        sel = np.nonzero(flat_e == e)[0]
        tok = sel >> 1
        n = len(sel)
        xt_e = np.zeros((D, C), dtype=BF16)
        xt_e[:, :n] = xt_bf[:, tok]
        glob[sel] = e * C + np.arange(n)
        in_maps.append({
            "xt": xt_e,
            "w1": np.ascontiguousarray(W1b[e]),
            "w2": np.ascontiguousarray(W2b[e]),
            # [128, m] with b[p, m] = bias[m*128 + p]
            "b1": np.ascontiguousarray(
                np.asarray(b1[e], dtype=np.float32).reshape(N_H, P).T),
            "b2": np.ascontiguousarray(
                np.asarray(b2[e], dtype=np.float32).reshape(N_O, P).T),
        })
    return in_maps, glob, flat_w, C, B, S


def _get_nc(C: int, iters: int = 1) -> bass.Bass:
    key = (MODE, C, iters)
    nc = _CACHE.get(key)
    if nc is None:
        build = _build_ilv if MODE == "ilv" else _build
        nc = _CACHE[key] = build(C, iters)
    return nc


def _combine(results, glob, flat_w, C, B, S):
    # yt arrives bf16 [O, C]; gate weights are applied here (host), so the
    # device never needs them and padding rows are simply never gathered.
    Y = np.stack([np.asarray(r["yt"]).astype(np.float32).T for r in results])
    Yflat = Y.reshape(E * C, O)
    out = (flat_w[0::2, None].astype(np.float32) * Yflat[glob[0::2]]
           + flat_w[1::2, None].astype(np.float32) * Yflat[glob[1::2]])
    return out.reshape(B, S, O).astype(np.float32, copy=False)


def kernel(x, Wg, W1, b1, W2, b2):
    in_maps, glob, flat_w, C, B, S = _prepare(x, Wg, W1, b1, W2, b2)
    nc = _get_nc(C)
    res = run_bass_kernel_spmd(nc, in_maps, core_ids=list(range(N_CORES)))
    return _combine(res.results, glob, flat_w, C, B, S)



# revision 11
# speedup vs baseline: 1.1571x; 1.1571x over previous
"""MoE layer (top-2 routing, 8 experts) for Trainium2 across 8 NeuronCores.

Strategy (expert-parallel):
  - The gate (logits = x @ Wg, top-2 + softmax) is computed on host; it is
    ~0.03% of the layer's FLOPs. Tokens are gathered per selected expert on
    host (the "all-to-all dispatch" of the sharding hint, done at input
    sharding time), padded to a common capacity C, one expert per core.
  - Each core runs its expert's FFN on its routed tokens:
        yT = (gelu(W1.T-tiled matmuls of x) @ W2 + b2) * gate_weight
    entirely in transposed [feature, token] layout so no on-chip transposes
    are needed. Matmul inputs are bf16 (f32 PSUM accumulation).
  - Host combines: out[t] = contrib(expert0(t)) + contrib(expert1(t)) via two
    vectorized gathers (gate weights were already applied on-device).

Per-core compute: 2 * C * D * H MACs (C = max expert count, ~1091 for the
reference routing) -> ~18.3 GFLOP -> ~295 us at the measured 2.0 GHz PE
clock; measured ~298 us/iteration on HW (~99% of that roofline).
"""

import os
from contextlib import ExitStack

import ml_dtypes
import numpy as np

import concourse.bass as bass
import concourse.tile as tile
from concourse import bacc, mybir
from concourse.bass_utils import run_bass_kernel_spmd

# Under axon without the NTFF hook module, trace=True would crash on import.
try:  # pragma: no cover
    import antenv.axon_hooks  # noqa: F401
except ImportError:
    os.environ.setdefault("BASS_NEVER_TRACE", "1")

BF16 = ml_dtypes.bfloat16
D, H, O, E, TOPK = 1024, 4096, 1024, 8, 2
P = 128
N_CORES = 8
N_D, N_H, N_O = D // P, H // P, O // P  # 8, 32, 8 k/m tiles

_CACHE: dict[int, bass.Bass] = {}


def _token_tiles(C):
    """Split capacity C into near-equal moving-dim chunks <= 512.

    Equal chunks beat a [512, ..., small-tail] split: a matmul's issue rate is
    max(N/2.4GHz, LDWEIGHTS ~107ns), so chunks below ~256 are LDW-bound and
    waste PE cycles. E.g. C=1152: 3x384 = ~246us vs [512,512,128] = ~273us.
    """
    n_chunks = -(-C // 512)
    base, rem = divmod(C, n_chunks)
    tiles, t0 = [], 0
    for i in range(n_chunks):
        n = base + (1 if i < rem else 0)
        tiles.append((t0, n))
        t0 += n
    return tiles


def _dedup_ldweights(nc, enabled: bool = True) -> int:
    """Remove InstLdweights whose weights-AP matches the previous PE weight
    load and which carry no sync info — the PE weight buffer already holds
    that tile, so the reload is a pure waste of PE issue slots.

    PE executes its instructions in block order; LDW/MM are the only PE
    instructions here and matmuls never modify the weight buffer.
    """
    if not enabled:
        return 0
    removed = 0
    for blk in nc.m.functions[0].blocks:
        last_key = None
        keep = []
        for inst in blk.instructions:
            if isinstance(inst, mybir.InstLdweights):
                si = inst.sync_info
                clean = si is None or (not si.on_wait and not si.on_update)
                key = (str(inst.ins[0]), str(inst.perf_mode),
                       str(inst.tile_position))
                if clean and key == last_key:
                    removed += 1
                    continue
                last_key = key
            elif isinstance(inst, mybir.InstMatmult):
                pass
            elif not isinstance(inst, (mybir.InstDMACopy, mybir.InstActivation,
                                       mybir.InstTensorTensor)):
                # control flow / drains / barriers: be conservative
                last_key = None
            keep.append(inst)
        blk.instructions[:] = keep
    return removed


def _build(C: int, iters: int = 1, degenerate_w: bool = False) -> bass.Bass:
    """One expert's FFN over C routed tokens, [feature, token] layout.

    Inputs (per core): xt [D, C] bf16 (tokens transposed), w1 [D, H] bf16,
    w2 [H, O] bf16, b1 [H] f32, b2 [O] f32. Output:
    yt [O, C] bf16 = gelu(x@w1+b1)@w2 + b2 (gate weights applied on host
    during combine; padding rows are never gathered there, so they need no
    zeroing on device).
    """
    f32, bf16 = mybir.dt.float32, mybir.dt.bfloat16
    nc = bacc.Bacc("TRN2", target_bir_lowering=False, debug=False,
                   num_devices=N_CORES)
    xt_d = nc.dram_tensor("xt", [D, C], bf16, kind="ExternalInput").ap()
    w1_d = nc.dram_tensor("w1", [D, H], bf16, kind="ExternalInput").ap()
    w2_d = nc.dram_tensor("w2", [H, O], bf16, kind="ExternalInput").ap()
    # b1/b2 arrive pre-transposed as [128, m] with b[p, m] = bias[m*128 + p]
    # (plain contiguous DMAs — fancy strided/broadcast DMA patterns fan out
    # across queues and blow the per-instruction sync-wait limit on their
    # first consumer).
    b1_d = nc.dram_tensor("b1", [P, N_H], f32, kind="ExternalInput").ap()
    b2_d = nc.dram_tensor("b2", [P, N_O], f32, kind="ExternalInput").ap()
    yt_d = nc.dram_tensor("yt", [O, C], bf16, kind="ExternalOutput").ap()

    with tile.TileContext(nc) as tc, ExitStack() as ctx:
        wpool = ctx.enter_context(tc.tile_pool(name="weights", bufs=1))
        xpool = ctx.enter_context(tc.tile_pool(name="xin", bufs=2))
        hpool = ctx.enter_context(tc.tile_pool(name="hts", bufs=34))
        ppool1 = ctx.enter_context(tc.tile_pool(name="ps1", bufs=4, space="PSUM"))
        ppool2 = ctx.enter_context(tc.tile_pool(name="ps2", bufs=4, space="PSUM"))
        # y DMAs have ~2us completion latency; with few bufs the
        # buffer-reuse dependency chains those latencies onto the critical
        # path (ACT -> PSUM -> PE backpressure). 8 bufs keeps them pipelined.
        ypool = ctx.enter_context(tc.tile_pool(name="yout", bufs=8))

        w1_sb = wpool.tile([P, N_D, H], bf16)   # 64 KB/partition
        w2_sb = wpool.tile([P, N_H, O], bf16)   # 64 KB/partition
        b1_sb = wpool.tile([P, N_H], f32)
        b2_sb = wpool.tile([P, N_O], f32)

        # DMA emission order == HWDGE queue order == consumption order:
        # tile-0 activations and biases first (the first matmuls need them),
        # then W1 chunked as phase A walks it, then W2 (first needed ~80us in).
        token_tiles = _token_tiles(C)
        xt_tiles = {}
        if iters == 1:
            (t0_first, nt_first) = token_tiles[0]
            xt_sb = xpool.tile([P, N_D, 512], bf16, tag="xt", name="xt0")
            for d in range(N_D):
                nc.sync.dma_start(
                    out=xt_sb[:, d, :nt_first],
                    in_=xt_d[d * P:(d + 1) * P, t0_first:t0_first + nt_first])
            xt_tiles[0] = xt_sb
        for hc in range(8):
            c0, c1 = hc * 512, (hc + 1) * 512
            for d in range(N_D):
                nc.sync.dma_start(out=w1_sb[:, d, c0:c1],
                                  in_=w1_d[d * P:(d + 1) * P, c0:c1])
            if hc == 0:
                # b1 is first read at the first gelu (~13us in); b2/g later.
                nc.sync.dma_start(out=b1_sb[:], in_=b1_d[:])
                nc.sync.dma_start(out=b2_sb[:], in_=b2_d[:])
        for h in range(N_H):
            nc.sync.dma_start(out=w2_sb[:, h, :],
                              in_=w2_d[h * P:(h + 1) * P, :])

        gelu = mybir.ActivationFunctionType.Gelu
        copy = mybir.ActivationFunctionType.Identity

        loop_ctx = ExitStack()
        if iters > 1:
            # timing-only variant: repeat the whole compute body on-device so
            # (wall(iters) - wall(1)) / (iters - 1) isolates HW exec time from
            # the axon dispatch/data-shipping overhead.
            loop_ctx.enter_context(tc.For_i(0, iters, 1))
        ctx.enter_context(loop_ctx)

        for it, (t0, nt) in enumerate(token_tiles):
            if it in xt_tiles:
                xt_sb = xt_tiles[it]
            else:
                xt_sb = xpool.tile([P, N_D, 512], bf16, tag="xt",
                                   name=f"xt{it}")
                for d in range(N_D):
                    nc.sync.dma_start(out=xt_sb[:, d, :nt],
                                      in_=xt_d[d * P:(d + 1) * P, t0:t0 + nt])
            # Phase A: hT[m*128:(m+1)*128, t] = gelu(x @ w1 + b1) per h-tile
            hts = []
            for m in range(N_H):
                ps = ppool1.tile([P, 512], f32, tag="ps1")
                for d in range(N_D):
                    lw = (w1_sb[:, 0, 0:P] if degenerate_w
                          else w1_sb[:, d, m * P:(m + 1) * P])
                    nc.tensor.matmul(ps[:, :nt], lhsT=lw,
                                     rhs=xt_sb[:, d, :nt],
                                     start=(d == 0), stop=(d == N_D - 1))
                ht = hpool.tile([P, 512], bf16, tag="ht")
                nc.scalar.activation(ht[:, :nt], ps[:, :nt], gelu,
                                     bias=b1_sb[:, m:m + 1])
                hts.append(ht)
            # Phase B: yT[o*128:(o+1)*128, t] = (hT.T-contraction @ w2 + b2)*g
            for o in range(N_O):
                ps2 = ppool2.tile([P, 512], f32, tag="ps2")
                for h in range(N_H):
                    lw = (w1_sb[:, 0, 0:P] if degenerate_w
                          else w2_sb[:, h, o * P:(o + 1) * P])
                    nc.tensor.matmul(ps2[:, :nt], lhsT=lw,
                                     rhs=hts[h][:, :nt],
                                     start=(h == 0), stop=(h == N_H - 1))
                yb = ypool.tile([P, 512], bf16, tag="yb")
                nc.scalar.activation(yb[:, :nt], ps2[:, :nt], copy,
                                     bias=b2_sb[:, o:o + 1])
                nc.sync.dma_start(out=yt_d[o * P:(o + 1) * P, t0:t0 + nt],
                                  in_=yb[:, :nt])
    nc.compile()
    return nc


def _build_ilv(C: int, iters: int = 1) -> bass.Bass:
    """Token-tile-interleaved variant: per stationary weight tile, the 3
    token tiles' matmuls are emitted back-to-back, so 2 of 3 LDWEIGHTS are
    adjacent duplicates and _dedup_ldweights removes them (~37ns of PE issue
    each). Costs: all token tiles' ht must be live at once (96KB/partition
    at nt=512), so w2 is NOT resident — it streams per o-block into a 2-slot
    window (2MB per o, ~6us, hidden under ~18us of per-o matmuls).
    Token tiles are capped at 448 so ht fits (3x28KB + w1 64KB + window 14KB
    + xt 21KB < 208KB/partition budget).
    """
    f32, bf16 = mybir.dt.float32, mybir.dt.bfloat16
    nc = bacc.Bacc("TRN2", target_bir_lowering=False, debug=False,
                   num_devices=N_CORES)
    xt_d = nc.dram_tensor("xt", [D, C], bf16, kind="ExternalInput").ap()
    w1_d = nc.dram_tensor("w1", [D, H], bf16, kind="ExternalInput").ap()
    w2_d = nc.dram_tensor("w2", [H, O], bf16, kind="ExternalInput").ap()
    b1_d = nc.dram_tensor("b1", [P, N_H], f32, kind="ExternalInput").ap()
    b2_d = nc.dram_tensor("b2", [P, N_O], f32, kind="ExternalInput").ap()
    yt_d = nc.dram_tensor("yt", [O, C], bf16, kind="ExternalOutput").ap()

    token_tiles = []
    n_chunks = -(-C // 448)
    base, rem = divmod(C, n_chunks)
    t0 = 0
    for i in range(n_chunks):
        n = base + (1 if i < rem else 0)
        token_tiles.append((t0, n))
        t0 += n
    ntt = len(token_tiles)
    nt_max = max(n for _, n in token_tiles)

    with tile.TileContext(nc) as tc, ExitStack() as ctx:
        wpool = ctx.enter_context(tc.tile_pool(name="weights", bufs=1))
        xpool = ctx.enter_context(tc.tile_pool(name="xin", bufs=ntt))
        hpool = ctx.enter_context(tc.tile_pool(name="hts", bufs=ntt))
        w2pool = ctx.enter_context(tc.tile_pool(name="w2win", bufs=2))
        ppool = ctx.enter_context(tc.tile_pool(name="ps", bufs=8, space="PSUM"))
        ypool = ctx.enter_context(tc.tile_pool(name="yout", bufs=8))

        w1_sb = wpool.tile([P, N_D, H], bf16)   # 64 KB/partition
        b1_sb = wpool.tile([P, N_H], f32)
        b2_sb = wpool.tile([P, N_O], f32)

        for hc in range(8):
            c0, c1 = hc * 512, (hc + 1) * 512
            for d in range(N_D):
                nc.sync.dma_start(out=w1_sb[:, d, c0:c1],
                                  in_=w1_d[d * P:(d + 1) * P, c0:c1])
            if hc == 0:
                nc.sync.dma_start(out=b1_sb[:], in_=b1_d[:])
                nc.sync.dma_start(out=b2_sb[:], in_=b2_d[:])

        gelu = mybir.ActivationFunctionType.Gelu
        copy = mybir.ActivationFunctionType.Identity

        loop_ctx = ExitStack()
        if iters > 1:
            loop_ctx.enter_context(tc.For_i(0, iters, 1))
        ctx.enter_context(loop_ctx)

        xts, hts = [], []
        for it, (t0, nt) in enumerate(token_tiles):
            xt_sb = xpool.tile([P, N_D, nt_max], bf16, tag="xt", name=f"xt{it}")
            for d in range(N_D):
                nc.sync.dma_start(out=xt_sb[:, d, :nt],
                                  in_=xt_d[d * P:(d + 1) * P, t0:t0 + nt])
            xts.append(xt_sb)
            hts.append(hpool.tile([P, N_H, nt_max], bf16, tag="ht",
                                  name=f"ht{it}"))
        # Phase A: hT = gelu(x @ w1 + b1), all token tiles interleaved
        for m in range(N_H):
            pss = [ppool.tile([P, 512], f32, tag="ps", name=f"psA{m}_{t}")
                   for t in range(ntt)]
            for d in range(N_D):
                lw = w1_sb[:, d, m * P:(m + 1) * P]
                for t, (t0, nt) in enumerate(token_tiles):
                    nc.tensor.matmul(pss[t][:, :nt], lhsT=lw,
                                     rhs=xts[t][:, d, :nt],
                                     start=(d == 0), stop=(d == N_D - 1))
            for t, (t0, nt) in enumerate(token_tiles):
                nc.scalar.activation(hts[t][:, m, :nt], pss[t][:, :nt], gelu,
                                     bias=b1_sb[:, m:m + 1])
        # Phase B: yT = hT.T-contraction @ w2 + b2; w2 streamed per o-block
        for o in range(N_O):
            w2blk = w2pool.tile([P, N_H, P], bf16, tag="w2")
            for h in range(N_H):
                nc.sync.dma_start(out=w2blk[:, h, :],
                                  in_=w2_d[h * P:(h + 1) * P,
                                           o * P:(o + 1) * P])
            pss = [ppool.tile([P, 512], f32, tag="ps", name=f"psB{o}_{t}")
                   for t in range(ntt)]
            for h in range(N_H):
                lw = w2blk[:, h, :]
                for t, (t0, nt) in enumerate(token_tiles):
                    nc.tensor.matmul(pss[t][:, :nt], lhsT=lw,
                                     rhs=hts[t][:, h, :nt],
                                     start=(h == 0), stop=(h == N_H - 1))
            for t, (t0, nt) in enumerate(token_tiles):
                yb = ypool.tile([P, 512], bf16, tag="yb")
                nc.scalar.activation(yb[:, :nt], pss[t][:, :nt], copy,
                                     bias=b2_sb[:, o:o + 1])
                nc.sync.dma_start(out=yt_d[o * P:(o + 1) * P, t0:t0 + nt],
                                  in_=yb[:, :nt])
    removed = _dedup_ldweights(nc)
    assert removed > 0, "expected adjacent duplicate LDWEIGHTS to dedup"
    nc.compile()
    return nc


MODE = os.environ.get("KMODE", "base")


def _prepare(x, Wg, W1, b1, W2, b2):
    """Host-side gating + per-expert gather. Returns (in_maps, glob, w, C, B, S)."""
    B, S, Dx = x.shape
    assert Dx == D and Wg.shape == (D, E), (x.shape, Wg.shape)
    T = B * S
    xf = np.ascontiguousarray(x.reshape(T, D), dtype=np.float32)
    logits = xf.astype(np.float64) @ Wg.astype(np.float64)
    top_i = np.argpartition(-logits, TOPK - 1, axis=1)[:, :TOPK]  # [T, 2]
    lv = np.take_along_axis(logits, top_i, axis=1)
    lv -= lv.max(axis=1, keepdims=True)
    ex = np.exp(lv)
    w = ex / ex.sum(axis=1, keepdims=True)  # [T, 2] softmax over the pair

    flat_e = top_i.reshape(-1)      # pair p = 2*t + k -> expert id
    flat_w = w.reshape(-1)
    counts = np.bincount(flat_e, minlength=E)
    # Tokens are the matmul free dim, so capacity needs no alignment at all;
    # every extra padded token costs PE time on all 8 cores.
    C = max(1024, int(counts.max()))

    xt_bf = np.ascontiguousarray(xf.T).astype(BF16)  # [D, T]
    W1b = W1.astype(BF16)
    W2b = W2.astype(BF16)

    in_maps = []
    glob = np.empty(2 * T, dtype=np.int64)  # pair -> row in stacked outputs
    for e in range(E):
        sel = np.nonzero(flat_e == e)[0]
        tok = sel >> 1
        n = len(sel)
        xt_e = np.zeros((D, C), dtype=BF16)
        xt_e[:, :n] = xt_bf[:, tok]
        glob[sel] = e * C + np.arange(n)
        in_maps.append({
            "xt": xt_e,
            "w1": np.ascontiguousarray(W1b[e]),
            "w2": np.ascontiguousarray(W2b[e]),
            # [128, m] with b[p, m] = bias[m*128 + p]
            "b1": np.ascontiguousarray(
                np.asarray(b1[e], dtype=np.float32).reshape(N_H, P).T),
            "b2": np.ascontiguousarray(
                np.asarray(b2[e], dtype=np.float32).reshape(N_O, P).T),
        })
    return in_maps, glob, flat_w, C, B, S


def _get_nc(C: int, iters: int = 1) -> bass.Bass:
    key = (MODE, C, iters)
    nc = _CACHE.get(key)
    if nc is None:
        build = _build_ilv if MODE == "ilv" else _build
        nc = _CACHE[key] = build(C, iters)
    return nc


def _combine(results, glob, flat_w, C, B, S):
    # yt arrives bf16 [O, C]; gate weights are applied here (host), so the
    # device never needs them and padding rows are simply never gathered.
    Y = np.stack([np.asarray(r["yt"]).astype(np.float32).T for r in results])
    Yflat = Y.reshape(E * C, O)
    out = (flat_w[0::2, None].astype(np.float32) * Yflat[glob[0::2]]
           + flat_w[1::2, None].astype(np.float32) * Yflat[glob[1::2]])
    return out.reshape(B, S, O).astype(np.float32, copy=False)


def kernel(x, Wg, W1, b1, W2, b2):
    in_maps, glob, flat_w, C, B, S = _prepare(x, Wg, W1, b1, W2, b2)
    nc = _get_nc(C)
    res = run_bass_kernel_spmd(nc, in_maps, core_ids=list(range(N_CORES)))
    return _combine(res.results, glob, flat_w, C, B, S)



# revision 13
# speedup vs baseline: 1.1620x; 1.0042x over previous
"""MoE layer (top-2 routing, 8 experts) for Trainium2 across 8 NeuronCores.

Strategy (expert-parallel):
  - The gate (logits = x @ Wg, top-2 + softmax) is computed on host; it is
    ~0.03% of the layer's FLOPs. Tokens are gathered per selected expert on
    host (the "all-to-all dispatch" of the sharding hint, done at input
    sharding time), padded to a common capacity C, one expert per core.
  - Each core runs its expert's FFN on its routed tokens:
        yT = (gelu(W1.T-tiled matmuls of x) @ W2 + b2) * gate_weight
    entirely in transposed [feature, token] layout so no on-chip transposes
    are needed. Matmul inputs are bf16 (f32 PSUM accumulation).
  - Host combines: out[t] = contrib(expert0(t)) + contrib(expert1(t)) via two
    vectorized gathers (gate weights were already applied on-device).

Per-core compute: 2 * C * D * H MACs (C = max expert count, ~1091 for the
reference routing) -> ~18.3 GFLOP -> ~295 us at the measured 2.0 GHz PE
clock; measured ~298 us/iteration on HW (~99% of that roofline).
"""

import os
from contextlib import ExitStack

import ml_dtypes
import numpy as np

import concourse.bass as bass
import concourse.tile as tile
from concourse import bacc, mybir
from concourse.bass_utils import run_bass_kernel_spmd

# Under axon without the NTFF hook module, trace=True would crash on import.
try:  # pragma: no cover
    import antenv.axon_hooks  # noqa: F401
except ImportError:
    os.environ.setdefault("BASS_NEVER_TRACE", "1")

BF16 = ml_dtypes.bfloat16
D, H, O, E, TOPK = 1024, 4096, 1024, 8, 2
P = 128
N_CORES = 8
N_D, N_H, N_O = D // P, H // P, O // P  # 8, 32, 8 k/m tiles

_CACHE: dict[int, bass.Bass] = {}


def _token_tiles(C):
    """Split capacity C into near-equal moving-dim chunks <= 512.

    Equal chunks beat a [512, ..., small-tail] split: a matmul's issue rate is
    max(N/2.4GHz, LDWEIGHTS ~107ns), so chunks below ~256 are LDW-bound and
    waste PE cycles. E.g. C=1152: 3x384 = ~246us vs [512,512,128] = ~273us.
    """
    n_chunks = -(-C // 512)
    base, rem = divmod(C, n_chunks)
    tiles, t0 = [], 0
    for i in range(n_chunks):
        n = base + (1 if i < rem else 0)
        tiles.append((t0, n))
        t0 += n
    return tiles


def _dedup_ldweights(nc, enabled: bool = True) -> int:
    """Remove InstLdweights whose weights-AP matches the previous PE weight
    load and which carry no sync info — the PE weight buffer already holds
    that tile, so the reload is a pure waste of PE issue slots.

    PE executes its instructions in block order; LDW/MM are the only PE
    instructions here and matmuls never modify the weight buffer.
    """
    if not enabled:
        return 0
    removed = 0
    for blk in nc.m.functions[0].blocks:
        last_key = None
        keep = []
        for inst in blk.instructions:
            if isinstance(inst, mybir.InstLdweights):
                si = inst.sync_info
                clean = si is None or (not si.on_wait and not si.on_update)
                key = (str(inst.ins[0]), str(inst.perf_mode),
                       str(inst.tile_position))
                if clean and key == last_key:
                    removed += 1
                    continue
                last_key = key
            elif isinstance(inst, mybir.InstMatmult):
                pass
            elif not isinstance(inst, (mybir.InstDMACopy, mybir.InstActivation,
                                       mybir.InstTensorTensor)):
                # control flow / drains / barriers: be conservative
                last_key = None
            keep.append(inst)
        blk.instructions[:] = keep
    return removed


def _build(C: int, iters: int = 1, degenerate_w: bool = False) -> bass.Bass:
    """One expert's FFN over C routed tokens, [feature, token] layout.

    Inputs (per core): xt [D, C] bf16 (tokens transposed), w1 [D, H] bf16,
    w2 [H, O] bf16, b1 [H] f32, b2 [O] f32. Output:
    yt [O, C] bf16 = gelu(x@w1+b1)@w2 + b2 (gate weights applied on host
    during combine; padding rows are never gathered there, so they need no
    zeroing on device).
    """
    f32, bf16 = mybir.dt.float32, mybir.dt.bfloat16
    nc = bacc.Bacc("TRN2", target_bir_lowering=False, debug=False,
                   num_devices=N_CORES)
    xt_d = nc.dram_tensor("xt", [D, C], bf16, kind="ExternalInput").ap()
    w1_d = nc.dram_tensor("w1", [D, H], bf16, kind="ExternalInput").ap()
    w2_d = nc.dram_tensor("w2", [H, O], bf16, kind="ExternalInput").ap()
    # b1/b2 arrive pre-transposed as [128, m] with b[p, m] = bias[m*128 + p]
    # (plain contiguous DMAs — fancy strided/broadcast DMA patterns fan out
    # across queues and blow the per-instruction sync-wait limit on their
    # first consumer).
    b1_d = nc.dram_tensor("b1", [P, N_H], f32, kind="ExternalInput").ap()
    b2_d = nc.dram_tensor("b2", [P, N_O], f32, kind="ExternalInput").ap()
    yt_d = nc.dram_tensor("yt", [O, C], bf16, kind="ExternalOutput").ap()

    with tile.TileContext(nc) as tc, ExitStack() as ctx:
        wpool = ctx.enter_context(tc.tile_pool(name="weights", bufs=1))
        xpool = ctx.enter_context(tc.tile_pool(name="xin", bufs=2))
        hpool = ctx.enter_context(tc.tile_pool(name="hts", bufs=34))
        ppool1 = ctx.enter_context(tc.tile_pool(name="ps1", bufs=4, space="PSUM"))
        ppool2 = ctx.enter_context(tc.tile_pool(name="ps2", bufs=4, space="PSUM"))
        # y DMAs have ~2us completion latency; with few bufs the
        # buffer-reuse dependency chains those latencies onto the critical
        # path (ACT -> PSUM -> PE backpressure). 8 bufs keeps them pipelined.
        ypool = ctx.enter_context(tc.tile_pool(name="yout", bufs=8))

        w1_sb = wpool.tile([P, N_D, H], bf16)   # 64 KB/partition
        w2_sb = wpool.tile([P, N_H, O], bf16)   # 64 KB/partition
        b1_sb = wpool.tile([P, N_H], f32)
        b2_sb = wpool.tile([P, N_O], f32)

        # DMA emission order == HWDGE queue order == consumption order:
        # tile-0 activations and biases first (the first matmuls need them),
        # then W1 chunked as phase A walks it, then W2 (first needed ~80us in).
        token_tiles = _token_tiles(C)
        xt_tiles = {}
        if iters == 1:
            (t0_first, nt_first) = token_tiles[0]
            xt_sb = xpool.tile([P, N_D, 512], bf16, tag="xt", name="xt0")
            for d in range(N_D):
                nc.sync.dma_start(
                    out=xt_sb[:, d, :nt_first],
                    in_=xt_d[d * P:(d + 1) * P, t0_first:t0_first + nt_first])
            xt_tiles[0] = xt_sb
        for hc in range(8):
            c0, c1 = hc * 512, (hc + 1) * 512
            for d in range(N_D):
                nc.sync.dma_start(out=w1_sb[:, d, c0:c1],
                                  in_=w1_d[d * P:(d + 1) * P, c0:c1])
            if hc == 0:
                # b1 is first read at the first gelu (~13us in); b2/g later.
                nc.sync.dma_start(out=b1_sb[:], in_=b1_d[:])
                nc.sync.dma_start(out=b2_sb[:], in_=b2_d[:])
        # w2 in o-major blocks: phase B walks o=0..7 (each o-block needs all
        # 32 h-tiles of one 128-col stripe), so this is consumption order for
        # the cold first pass; irrelevant in the For_i steady state.
        for o in range(N_O):
            for h in range(N_H):
                nc.sync.dma_start(out=w2_sb[:, h, o * P:(o + 1) * P],
                                  in_=w2_d[h * P:(h + 1) * P,
                                           o * P:(o + 1) * P])

        gelu = mybir.ActivationFunctionType.Gelu
        copy = mybir.ActivationFunctionType.Identity

        loop_ctx = ExitStack()
        if iters > 1:
            # timing-only variant: repeat the whole compute body on-device so
            # (wall(iters) - wall(1)) / (iters - 1) isolates HW exec time from
            # the axon dispatch/data-shipping overhead.
            loop_ctx.enter_context(tc.For_i(0, iters, 1))
        ctx.enter_context(loop_ctx)

        for it, (t0, nt) in enumerate(token_tiles):
            if it in xt_tiles:
                xt_sb = xt_tiles[it]
            else:
                xt_sb = xpool.tile([P, N_D, 512], bf16, tag="xt",
                                   name=f"xt{it}")
                for d in range(N_D):
                    nc.sync.dma_start(out=xt_sb[:, d, :nt],
                                      in_=xt_d[d * P:(d + 1) * P, t0:t0 + nt])
            # Phase A: hT[m*128:(m+1)*128, t] = gelu(x @ w1 + b1) per h-tile
            hts = []
            for m in range(N_H):
                ps = ppool1.tile([P, 512], f32, tag="ps1")
                for d in range(N_D):
                    lw = (w1_sb[:, 0, 0:P] if degenerate_w
                          else w1_sb[:, d, m * P:(m + 1) * P])
                    nc.tensor.matmul(ps[:, :nt], lhsT=lw,
                                     rhs=xt_sb[:, d, :nt],
                                     start=(d == 0), stop=(d == N_D - 1))
                ht = hpool.tile([P, 512], bf16, tag="ht")
                nc.scalar.activation(ht[:, :nt], ps[:, :nt], gelu,
                                     bias=b1_sb[:, m:m + 1])
                hts.append(ht)
            # Phase B: yT[o*128:(o+1)*128, t] = (hT.T-contraction @ w2 + b2)*g
            for o in range(N_O):
                ps2 = ppool2.tile([P, 512], f32, tag="ps2")
                for h in range(N_H):
                    lw = (w1_sb[:, 0, 0:P] if degenerate_w
                          else w2_sb[:, h, o * P:(o + 1) * P])
                    nc.tensor.matmul(ps2[:, :nt], lhsT=lw,
                                     rhs=hts[h][:, :nt],
                                     start=(h == 0), stop=(h == N_H - 1))
                yb = ypool.tile([P, 512], bf16, tag="yb")
                # drain on DVE (per-partition scalar add of b2), keeping the
                # scalar engine free for phase A gelus
                nc.vector.tensor_scalar_add(yb[:, :nt], ps2[:, :nt],
                                            b2_sb[:, o:o + 1])
                nc.sync.dma_start(out=yt_d[o * P:(o + 1) * P, t0:t0 + nt],
                                  in_=yb[:, :nt])
    nc.compile()
    return nc


def _build_ilv(C: int, iters: int = 1) -> bass.Bass:
    """Token-tile-interleaved variant: per stationary weight tile, the 3
    token tiles' matmuls are emitted back-to-back, so 2 of 3 LDWEIGHTS are
    adjacent duplicates and _dedup_ldweights removes them (~37ns of PE issue
    each). Costs: all token tiles' ht must be live at once (96KB/partition
    at nt=512), so w2 is NOT resident — it streams per o-block into a 2-slot
    window (2MB per o, ~6us, hidden under ~18us of per-o matmuls).
    Token tiles are capped at 448 so ht fits (3x28KB + w1 64KB + window 14KB
    + xt 21KB < 208KB/partition budget).
    """
    f32, bf16 = mybir.dt.float32, mybir.dt.bfloat16
    nc = bacc.Bacc("TRN2", target_bir_lowering=False, debug=False,
                   num_devices=N_CORES)
    xt_d = nc.dram_tensor("xt", [D, C], bf16, kind="ExternalInput").ap()
    w1_d = nc.dram_tensor("w1", [D, H], bf16, kind="ExternalInput").ap()
    w2_d = nc.dram_tensor("w2", [H, O], bf16, kind="ExternalInput").ap()
    b1_d = nc.dram_tensor("b1", [P, N_H], f32, kind="ExternalInput").ap()
    b2_d = nc.dram_tensor("b2", [P, N_O], f32, kind="ExternalInput").ap()
    yt_d = nc.dram_tensor("yt", [O, C], bf16, kind="ExternalOutput").ap()

    token_tiles = []
    n_chunks = -(-C // 448)
    base, rem = divmod(C, n_chunks)
    t0 = 0
    for i in range(n_chunks):
        n = base + (1 if i < rem else 0)
        token_tiles.append((t0, n))
        t0 += n
    ntt = len(token_tiles)
    nt_max = max(n for _, n in token_tiles)

    with tile.TileContext(nc) as tc, ExitStack() as ctx:
        wpool = ctx.enter_context(tc.tile_pool(name="weights", bufs=1))
        xpool = ctx.enter_context(tc.tile_pool(name="xin", bufs=ntt))
        hpool = ctx.enter_context(tc.tile_pool(name="hts", bufs=ntt))
        w2pool = ctx.enter_context(tc.tile_pool(name="w2win", bufs=2))
        ppool = ctx.enter_context(tc.tile_pool(name="ps", bufs=8, space="PSUM"))
        ypool = ctx.enter_context(tc.tile_pool(name="yout", bufs=8))

        w1_sb = wpool.tile([P, N_D, H], bf16)   # 64 KB/partition
        b1_sb = wpool.tile([P, N_H], f32)
        b2_sb = wpool.tile([P, N_O], f32)

        for hc in range(8):
            c0, c1 = hc * 512, (hc + 1) * 512
            for d in range(N_D):
                nc.sync.dma_start(out=w1_sb[:, d, c0:c1],
                                  in_=w1_d[d * P:(d + 1) * P, c0:c1])
            if hc == 0:
                nc.sync.dma_start(out=b1_sb[:], in_=b1_d[:])
                nc.sync.dma_start(out=b2_sb[:], in_=b2_d[:])

        gelu = mybir.ActivationFunctionType.Gelu
        copy = mybir.ActivationFunctionType.Identity

        loop_ctx = ExitStack()
        if iters > 1:
            loop_ctx.enter_context(tc.For_i(0, iters, 1))
        ctx.enter_context(loop_ctx)

        xts, hts = [], []
        for it, (t0, nt) in enumerate(token_tiles):
            xt_sb = xpool.tile([P, N_D, nt_max], bf16, tag="xt", name=f"xt{it}")
            for d in range(N_D):
                nc.sync.dma_start(out=xt_sb[:, d, :nt],
                                  in_=xt_d[d * P:(d + 1) * P, t0:t0 + nt])
            xts.append(xt_sb)
            hts.append(hpool.tile([P, N_H, nt_max], bf16, tag="ht",
                                  name=f"ht{it}"))
        # Phase A: hT = gelu(x @ w1 + b1), all token tiles interleaved
        for m in range(N_H):
            pss = [ppool.tile([P, 512], f32, tag="ps", name=f"psA{m}_{t}")
                   for t in range(ntt)]
            for d in range(N_D):
                lw = w1_sb[:, d, m * P:(m + 1) * P]
                for t, (t0, nt) in enumerate(token_tiles):
                    nc.tensor.matmul(pss[t][:, :nt], lhsT=lw,
                                     rhs=xts[t][:, d, :nt],
                                     start=(d == 0), stop=(d == N_D - 1))
            for t, (t0, nt) in enumerate(token_tiles):
                nc.scalar.activation(hts[t][:, m, :nt], pss[t][:, :nt], gelu,
                                     bias=b1_sb[:, m:m + 1])
        # Phase B: yT = hT.T-contraction @ w2 + b2; w2 streamed per o-block
        for o in range(N_O):
            w2blk = w2pool.tile([P, N_H, P], bf16, tag="w2")
            for h in range(N_H):
                nc.sync.dma_start(out=w2blk[:, h, :],
                                  in_=w2_d[h * P:(h + 1) * P,
                                           o * P:(o + 1) * P])
            pss = [ppool.tile([P, 512], f32, tag="ps", name=f"psB{o}_{t}")
                   for t in range(ntt)]
            for h in range(N_H):
                lw = w2blk[:, h, :]
                for t, (t0, nt) in enumerate(token_tiles):
                    nc.tensor.matmul(pss[t][:, :nt], lhsT=lw,
                                     rhs=hts[t][:, h, :nt],
                                     start=(h == 0), stop=(h == N_H - 1))
            for t, (t0, nt) in enumerate(token_tiles):
                yb = ypool.tile([P, 512], bf16, tag="yb")
                nc.scalar.activation(yb[:, :nt], pss[t][:, :nt], copy,
                                     bias=b2_sb[:, o:o + 1])
                nc.sync.dma_start(out=yt_d[o * P:(o + 1) * P, t0:t0 + nt],
                                  in_=yb[:, :nt])
    removed = _dedup_ldweights(nc)
    assert removed > 0, "expected adjacent duplicate LDWEIGHTS to dedup"
    nc.compile()
    return nc


MODE = os.environ.get("KMODE", "base")


def _prepare(x, Wg, W1, b1, W2, b2):
    """Host-side gating + per-expert gather. Returns (in_maps, glob, w, C, B, S)."""
    B, S, Dx = x.shape
    assert Dx == D and Wg.shape == (D, E), (x.shape, Wg.shape)
    T = B * S
    xf = np.ascontiguousarray(x.reshape(T, D), dtype=np.float32)
    logits = xf.astype(np.float64) @ Wg.astype(np.float64)
    top_i = np.argpartition(-logits, TOPK - 1, axis=1)[:, :TOPK]  # [T, 2]
    lv = np.take_along_axis(logits, top_i, axis=1)
    lv -= lv.max(axis=1, keepdims=True)
    ex = np.exp(lv)
    w = ex / ex.sum(axis=1, keepdims=True)  # [T, 2] softmax over the pair

    flat_e = top_i.reshape(-1)      # pair p = 2*t + k -> expert id
    flat_w = w.reshape(-1)
    counts = np.bincount(flat_e, minlength=E)
    # Tokens are the matmul free dim, so capacity needs no alignment at all;
    # every extra padded token costs PE time on all 8 cores.
    C = max(1024, int(counts.max()))

    xt_bf = np.ascontiguousarray(xf.T).astype(BF16)  # [D, T]
    W1b = W1.astype(BF16)
    W2b = W2.astype(BF16)

    in_maps = []
    glob = np.empty(2 * T, dtype=np.int64)  # pair -> row in stacked outputs
    for e in range(E):
        sel = np.nonzero(flat_e == e)[0]
        tok = sel >> 1
        n = len(sel)
        xt_e = np.zeros((D, C), dtype=BF16)
        xt_e[:, :n] = xt_bf[:, tok]
        glob[sel] = e * C + np.arange(n)
        in_maps.append({
            "xt": xt_e,
            "w1": np.ascontiguousarray(W1b[e]),
            "w2": np.ascontiguousarray(W2b[e]),
            # [128, m] with b[p, m] = bias[m*128 + p]
            "b1": np.ascontiguousarray(
                np.asarray(b1[e], dtype=np.float32).reshape(N_H, P).T),
            "b2": np.ascontiguousarray(
                np.asarray(b2[e], dtype=np.float32).reshape(N_O, P).T),
        })
    return in_maps, glob, flat_w, C, B, S


def _get_nc(C: int, iters: int = 1) -> bass.Bass:
    key = (MODE, C, iters)
    nc = _CACHE.get(key)
    if nc is None:
        build = _build_ilv if MODE == "ilv" else _build
        nc = _CACHE[key] = build(C, iters)
    return nc


def _combine(results, glob, flat_w, C, B, S):
    # yt arrives bf16 [O, C]; gate weights are applied here (host), so the
    # device never needs them and padding rows are simply never gathered.
    Y = np.stack([np.asarray(r["yt"]).astype(np.float32).T for r in results])
    Yflat = Y.reshape(E * C, O)
    out = (flat_w[0::2, None].astype(np.float32) * Yflat[glob[0::2]]
           + flat_w[1::2, None].astype(np.float32) * Yflat[glob[1::2]])
    return out.reshape(B, S, O).astype(np.float32, copy=False)


def kernel(x, Wg, W1, b1, W2, b2):
    in_maps, glob, flat_w, C, B, S = _prepare(x, Wg, W1, b1, W2, b2)
    nc = _get_nc(C)
    res = run_bass_kernel_spmd(nc, in_maps, core_ids=list(range(N_CORES)))
    return _combine(res.results, glob, flat_w, C, B, S)



# revision 16
# speedup vs baseline: 1.1703x; 1.0072x over previous
"""MoE layer (top-2 routing, 8 experts) for Trainium2 across 8 NeuronCores.

Strategy (expert-parallel):
  - The gate (logits = x @ Wg, top-2 + softmax) is computed on host; it is
    ~0.03% of the layer's FLOPs. Tokens are gathered per selected expert on
    host (the "all-to-all dispatch" of the sharding hint, done at input
    sharding time), padded to a common capacity C, one expert per core.
  - Each core runs its expert's FFN on its routed tokens:
        yT = (gelu(W1.T-tiled matmuls of x) @ W2 + b2) * gate_weight
    entirely in transposed [feature, token] layout so no on-chip transposes
    are needed. Matmul inputs are bf16 (f32 PSUM accumulation).
  - Host combines: out[t] = contrib(expert0(t)) + contrib(expert1(t)) via two
    vectorized gathers (gate weights were already applied on-device).

Per-core compute: 2 * C * D * H MACs (C = max expert count, ~1091 for the
reference routing) -> ~18.3 GFLOP -> ~295 us at the measured 2.0 GHz PE
clock; measured ~298 us/iteration on HW (~99% of that roofline).
"""

import os
from contextlib import ExitStack

import ml_dtypes
import numpy as np

import concourse.bass as bass
import concourse.tile as tile
from concourse import bacc, mybir
from concourse.bass_utils import run_bass_kernel_spmd

# Under axon without the NTFF hook module, trace=True would crash on import.
try:  # pragma: no cover
    import antenv.axon_hooks  # noqa: F401
except ImportError:
    os.environ.setdefault("BASS_NEVER_TRACE", "1")

BF16 = ml_dtypes.bfloat16
D, H, O, E, TOPK = 1024, 4096, 1024, 8, 2
P = 128
N_CORES = 8
N_D, N_H, N_O = D // P, H // P, O // P  # 8, 32, 8 k/m tiles

_CACHE: dict[int, bass.Bass] = {}


def _token_tiles(C):
    """Split capacity C into near-equal moving-dim chunks <= 512.

    Equal chunks beat a [512, ..., small-tail] split: a matmul's issue rate is
    max(N/2.4GHz, LDWEIGHTS ~107ns), so chunks below ~256 are LDW-bound and
    waste PE cycles. E.g. C=1152: 3x384 = ~246us vs [512,512,128] = ~273us.
    """
    n_chunks = -(-C // 512)
    base, rem = divmod(C, n_chunks)
    tiles, t0 = [], 0
    for i in range(n_chunks):
        n = base + (1 if i < rem else 0)
        tiles.append((t0, n))
        t0 += n
    return tiles


def _dedup_ldweights(nc, enabled: bool = True) -> int:
    """Remove InstLdweights whose weights-AP matches the previous PE weight
    load and which carry no sync info — the PE weight buffer already holds
    that tile, so the reload is a pure waste of PE issue slots.

    PE executes its instructions in block order; LDW/MM are the only PE
    instructions here and matmuls never modify the weight buffer.
    """
    if not enabled:
        return 0
    removed = 0
    for blk in nc.m.functions[0].blocks:
        last_key = None
        keep = []
        for inst in blk.instructions:
            if isinstance(inst, mybir.InstLdweights):
                si = inst.sync_info
                clean = si is None or (not si.on_wait and not si.on_update)
                key = (str(inst.ins[0]), str(inst.perf_mode),
                       str(inst.tile_position))
                if clean and key == last_key:
                    removed += 1
                    continue
                last_key = key
            elif isinstance(inst, mybir.InstMatmult):
                pass
            elif not isinstance(inst, (mybir.InstDMACopy, mybir.InstActivation,
                                       mybir.InstTensorTensor)):
                # control flow / drains / barriers: be conservative
                last_key = None
            keep.append(inst)
        blk.instructions[:] = keep
    return removed


def _build(C: int, iters: int = 1, degenerate_w: bool = False) -> bass.Bass:
    """One expert's FFN over C routed tokens, [feature, token] layout.

    Inputs (per core): xt [D, C] bf16 (tokens transposed), w1 [D, H] bf16,
    w2 [H, O] bf16, b1 [H] f32, b2 [O] f32. Output:
    yt [O, C] bf16 = gelu(x@w1+b1)@w2 + b2 (gate weights applied on host
    during combine; padding rows are never gathered there, so they need no
    zeroing on device).
    """
    f32, bf16 = mybir.dt.float32, mybir.dt.bfloat16
    nc = bacc.Bacc("TRN2", target_bir_lowering=False, debug=False,
                   num_devices=N_CORES)
    xt_d = nc.dram_tensor("xt", [D, C], bf16, kind="ExternalInput").ap()
    w1_d = nc.dram_tensor("w1", [D, H], bf16, kind="ExternalInput").ap()
    w2_d = nc.dram_tensor("w2", [H, O], bf16, kind="ExternalInput").ap()
    # b1/b2 arrive pre-transposed as [128, m] with b[p, m] = bias[m*128 + p]
    # (plain contiguous DMAs — fancy strided/broadcast DMA patterns fan out
    # across queues and blow the per-instruction sync-wait limit on their
    # first consumer).
    b1_d = nc.dram_tensor("b1", [P, N_H], f32, kind="ExternalInput").ap()
    b2_d = nc.dram_tensor("b2", [P, N_O], f32, kind="ExternalInput").ap()
    yt_d = nc.dram_tensor("yt", [O, C], bf16, kind="ExternalOutput").ap()

    with tile.TileContext(nc) as tc, ExitStack() as ctx:
        wpool = ctx.enter_context(tc.tile_pool(name="weights", bufs=1))
        xpool = ctx.enter_context(tc.tile_pool(name="xin", bufs=2))
        hpool = ctx.enter_context(tc.tile_pool(name="hts", bufs=34))
        ppool1 = ctx.enter_context(tc.tile_pool(name="ps1", bufs=4, space="PSUM"))
        ppool2 = ctx.enter_context(tc.tile_pool(name="ps2", bufs=4, space="PSUM"))
        # y DMAs have ~2us completion latency; with few bufs the
        # buffer-reuse dependency chains those latencies onto the critical
        # path (ACT -> PSUM -> PE backpressure). 8 bufs keeps them pipelined.
        ypool = ctx.enter_context(tc.tile_pool(name="yout", bufs=8))

        w1_sb = wpool.tile([P, N_D, H], bf16)   # 64 KB/partition
        w2_sb = wpool.tile([P, N_H, O], bf16)   # 64 KB/partition
        b1_sb = wpool.tile([P, N_H], f32)
        b2_sb = wpool.tile([P, N_O], f32)

        # DMA emission order == HWDGE queue order == consumption order:
        # tile-0 activations and biases first (the first matmuls need them),
        # then W1 chunked as phase A walks it, then W2 (first needed ~80us in).
        token_tiles = _token_tiles(C)
        xt_tiles = {}
        if iters == 1:
            (t0_first, nt_first) = token_tiles[0]
            xt_sb = xpool.tile([P, N_D, 512], bf16, tag="xt", name="xt0")
            for d in range(N_D):
                nc.sync.dma_start(
                    out=xt_sb[:, d, :nt_first],
                    in_=xt_d[d * P:(d + 1) * P, t0_first:t0_first + nt_first])
            xt_tiles[0] = xt_sb
        for hc in range(8):
            c0, c1 = hc * 512, (hc + 1) * 512
            for d in range(N_D):
                nc.sync.dma_start(out=w1_sb[:, d, c0:c1],
                                  in_=w1_d[d * P:(d + 1) * P, c0:c1])
            if hc == 0:
                # b1 is first read at the first gelu (~13us in); b2/g later.
                nc.sync.dma_start(out=b1_sb[:], in_=b1_d[:])
                nc.sync.dma_start(out=b2_sb[:], in_=b2_d[:])
        # w2 in o-major blocks: phase B walks o=0..7 (each o-block needs all
        # 32 h-tiles of one 128-col stripe), so this is consumption order for
        # the cold first pass; irrelevant in the For_i steady state.
        for o in range(N_O):
            for h in range(N_H):
                nc.sync.dma_start(out=w2_sb[:, h, o * P:(o + 1) * P],
                                  in_=w2_d[h * P:(h + 1) * P,
                                           o * P:(o + 1) * P])

        gelu = mybir.ActivationFunctionType.Gelu
        copy = mybir.ActivationFunctionType.Identity

        loop_ctx = ExitStack()
        if iters > 1:
            # timing-only variant: repeat the whole compute body on-device so
            # (wall(iters) - wall(1)) / (iters - 1) isolates HW exec time from
            # the axon dispatch/data-shipping overhead.
            loop_ctx.enter_context(tc.For_i(0, iters, 1))
        ctx.enter_context(loop_ctx)

        for it, (t0, nt) in enumerate(token_tiles):
            if it in xt_tiles:
                xt_sb = xt_tiles[it]
            else:
                xt_sb = xpool.tile([P, N_D, 512], bf16, tag="xt",
                                   name=f"xt{it}")
                for d in range(N_D):
                    nc.sync.dma_start(out=xt_sb[:, d, :nt],
                                      in_=xt_d[d * P:(d + 1) * P, t0:t0 + nt])
            # Phase A: hT[m*128:(m+1)*128, t] = gelu(x @ w1 + b1) per h-tile
            hts = []
            for m in range(N_H):
                ps = ppool1.tile([P, 512], f32, tag="ps1")
                for d in range(N_D):
                    lw = (w1_sb[:, 0, 0:P] if degenerate_w
                          else w1_sb[:, d, m * P:(m + 1) * P])
                    nc.tensor.matmul(ps[:, :nt], lhsT=lw,
                                     rhs=xt_sb[:, d, :nt],
                                     start=(d == 0), stop=(d == N_D - 1))
                ht = hpool.tile([P, 512], bf16, tag="ht")
                nc.scalar.activation(ht[:, :nt], ps[:, :nt], gelu,
                                     bias=b1_sb[:, m:m + 1])
                hts.append(ht)
            # Phase B: yT[o*128:(o+1)*128, t] = (hT.T-contraction @ w2 + b2)*g
            for o in range(N_O):
                ps2 = ppool2.tile([P, 512], f32, tag="ps2")
                for h in range(N_H):
                    lw = (w1_sb[:, 0, 0:P] if degenerate_w
                          else w2_sb[:, h, o * P:(o + 1) * P])
                    nc.tensor.matmul(ps2[:, :nt], lhsT=lw,
                                     rhs=hts[h][:, :nt],
                                     start=(h == 0), stop=(h == N_H - 1))
                yb = ypool.tile([P, 512], bf16, tag="yb")
                # drain on DVE (per-partition scalar add of b2), keeping the
                # scalar engine free for phase A gelus
                nc.vector.tensor_scalar_add(yb[:, :nt], ps2[:, :nt],
                                            b2_sb[:, o:o + 1])
                nc.sync.dma_start(out=yt_d[o * P:(o + 1) * P, t0:t0 + nt],
                                  in_=yb[:, :nt])
    nc.compile()
    return nc


def _build_ilv(C: int, iters: int = 1) -> bass.Bass:
    """Token-tile-interleaved variant: per stationary weight tile, the 3
    token tiles' matmuls are emitted back-to-back, so 2 of 3 LDWEIGHTS are
    adjacent duplicates and _dedup_ldweights removes them (~37ns of PE issue
    each). Costs: all token tiles' ht must be live at once (96KB/partition
    at nt=512), so w2 is NOT resident — it streams per o-block into a 2-slot
    window (2MB per o, ~6us, hidden under ~18us of per-o matmuls).
    Token tiles are capped at 448 so ht fits (3x28KB + w1 64KB + window 14KB
    + xt 21KB < 208KB/partition budget).
    """
    f32, bf16 = mybir.dt.float32, mybir.dt.bfloat16
    nc = bacc.Bacc("TRN2", target_bir_lowering=False, debug=False,
                   num_devices=N_CORES)
    xt_d = nc.dram_tensor("xt", [D, C], bf16, kind="ExternalInput").ap()
    w1_d = nc.dram_tensor("w1", [D, H], bf16, kind="ExternalInput").ap()
    w2_d = nc.dram_tensor("w2", [H, O], bf16, kind="ExternalInput").ap()
    b1_d = nc.dram_tensor("b1", [P, N_H], f32, kind="ExternalInput").ap()
    b2_d = nc.dram_tensor("b2", [P, N_O], f32, kind="ExternalInput").ap()
    yt_d = nc.dram_tensor("yt", [O, C], bf16, kind="ExternalOutput").ap()

    token_tiles = []
    n_chunks = -(-C // 448)
    base, rem = divmod(C, n_chunks)
    t0 = 0
    for i in range(n_chunks):
        n = base + (1 if i < rem else 0)
        token_tiles.append((t0, n))
        t0 += n
    ntt = len(token_tiles)
    nt_max = max(n for _, n in token_tiles)

    with tile.TileContext(nc) as tc, ExitStack() as ctx:
        wpool = ctx.enter_context(tc.tile_pool(name="weights", bufs=1))
        xpool = ctx.enter_context(tc.tile_pool(name="xin", bufs=ntt))
        hpool = ctx.enter_context(tc.tile_pool(name="hts", bufs=ntt))
        w2pool = ctx.enter_context(tc.tile_pool(name="w2win", bufs=2))
        ppool = ctx.enter_context(tc.tile_pool(name="ps", bufs=8, space="PSUM"))
        ypool = ctx.enter_context(tc.tile_pool(name="yout", bufs=8))

        w1_sb = wpool.tile([P, N_D, H], bf16)   # 64 KB/partition
        b1_sb = wpool.tile([P, N_H], f32)
        b2_sb = wpool.tile([P, N_O], f32)

        for hc in range(8):
            c0, c1 = hc * 512, (hc + 1) * 512
            for d in range(N_D):
                nc.sync.dma_start(out=w1_sb[:, d, c0:c1],
                                  in_=w1_d[d * P:(d + 1) * P, c0:c1])
            if hc == 0:
                nc.sync.dma_start(out=b1_sb[:], in_=b1_d[:])
                nc.sync.dma_start(out=b2_sb[:], in_=b2_d[:])

        gelu = mybir.ActivationFunctionType.Gelu
        copy = mybir.ActivationFunctionType.Identity

        loop_ctx = ExitStack()
        if iters > 1:
            loop_ctx.enter_context(tc.For_i(0, iters, 1))
        ctx.enter_context(loop_ctx)

        xts, hts = [], []
        for it, (t0, nt) in enumerate(token_tiles):
            xt_sb = xpool.tile([P, N_D, nt_max], bf16, tag="xt", name=f"xt{it}")
            for d in range(N_D):
                nc.sync.dma_start(out=xt_sb[:, d, :nt],
                                  in_=xt_d[d * P:(d + 1) * P, t0:t0 + nt])
            xts.append(xt_sb)
            hts.append(hpool.tile([P, N_H, nt_max], bf16, tag="ht",
                                  name=f"ht{it}"))
        # Phase A: hT = gelu(x @ w1 + b1), all token tiles interleaved
        for m in range(N_H):
            pss = [ppool.tile([P, 512], f32, tag="ps", name=f"psA{m}_{t}")
                   for t in range(ntt)]
            for d in range(N_D):
                lw = w1_sb[:, d, m * P:(m + 1) * P]
                for t, (t0, nt) in enumerate(token_tiles):
                    nc.tensor.matmul(pss[t][:, :nt], lhsT=lw,
                                     rhs=xts[t][:, d, :nt],
                                     start=(d == 0), stop=(d == N_D - 1))
            for t, (t0, nt) in enumerate(token_tiles):
                nc.scalar.activation(hts[t][:, m, :nt], pss[t][:, :nt], gelu,
                                     bias=b1_sb[:, m:m + 1])
        # Phase B: yT = hT.T-contraction @ w2 + b2; w2 streamed per o-block
        for o in range(N_O):
            w2blk = w2pool.tile([P, N_H, P], bf16, tag="w2")
            for h in range(N_H):
                nc.sync.dma_start(out=w2blk[:, h, :],
                                  in_=w2_d[h * P:(h + 1) * P,
                                           o * P:(o + 1) * P])
            pss = [ppool.tile([P, 512], f32, tag="ps", name=f"psB{o}_{t}")
                   for t in range(ntt)]
            for h in range(N_H):
                lw = w2blk[:, h, :]
                for t, (t0, nt) in enumerate(token_tiles):
                    nc.tensor.matmul(pss[t][:, :nt], lhsT=lw,
                                     rhs=hts[t][:, h, :nt],
                                     start=(h == 0), stop=(h == N_H - 1))
            for t, (t0, nt) in enumerate(token_tiles):
                yb = ypool.tile([P, 512], bf16, tag="yb")
                nc.scalar.activation(yb[:, :nt], pss[t][:, :nt], copy,
                                     bias=b2_sb[:, o:o + 1])
                nc.sync.dma_start(out=yt_d[o * P:(o + 1) * P, t0:t0 + nt],
                                  in_=yb[:, :nt])
    removed = _dedup_ldweights(nc)
    assert removed > 0, "expected adjacent duplicate LDWEIGHTS to dedup"
    nc.compile()
    return nc


def _build_ilv2(C: int, iters: int = 1) -> bass.Bass:
    """Pairwise token-tile interleave, both weight matrices resident.

    Empirically the PE serializes each FWL LDWEIGHTS (~53ns) with the
    following matmul instead of hiding it (199ns/MM measured vs 152ns
    streaming at 2.4GHz for nt=364). Emitting the two paired token tiles'
    matmuls back-to-back under one weight tile lets _dedup_ldweights drop
    every second LDWEIGHTS: 1536 -> 1024 per pass (~-25us).
    Token tiles: [364, 364, 363]; (t0, t1) interleaved, t2 solo, phases
    A/B per group so only the group's ht tiles are live (w1 64K + w2 64K +
    ht 45.5K + xt 17K stays under the ~208KB/partition budget).
    """
    f32, bf16 = mybir.dt.float32, mybir.dt.bfloat16
    nc = bacc.Bacc("TRN2", target_bir_lowering=False, debug=False,
                   num_devices=N_CORES)
    xt_d = nc.dram_tensor("xt", [D, C], bf16, kind="ExternalInput").ap()
    w1_d = nc.dram_tensor("w1", [D, H], bf16, kind="ExternalInput").ap()
    w2_d = nc.dram_tensor("w2", [H, O], bf16, kind="ExternalInput").ap()
    b1_d = nc.dram_tensor("b1", [P, N_H], f32, kind="ExternalInput").ap()
    b2_d = nc.dram_tensor("b2", [P, N_O], f32, kind="ExternalInput").ap()
    yt_d = nc.dram_tensor("yt", [O, C], bf16, kind="ExternalOutput").ap()

    token_tiles = _token_tiles(C)
    nt_max = max(n for _, n in token_tiles)
    # pair tiles greedily: [(0,1), (2,)] for 3 tiles
    tile_groups = [tuple(range(i, min(i + 2, len(token_tiles))))
                   for i in range(0, len(token_tiles), 2)]

    with tile.TileContext(nc) as tc, ExitStack() as ctx:
        # Only the interleaved pair must be live at once; the solo tile
        # cycles the pair's buffers (its allocation waits for the prior
        # group's phase B, which precedes it on the PE anyway).
        wpool = ctx.enter_context(tc.tile_pool(name="weights", bufs=1))
        xpool = ctx.enter_context(tc.tile_pool(name="xin", bufs=2))
        hpool = ctx.enter_context(tc.tile_pool(name="hts", bufs=2))
        ppool1 = ctx.enter_context(tc.tile_pool(name="ps1", bufs=4, space="PSUM"))
        ppool2 = ctx.enter_context(tc.tile_pool(name="ps2", bufs=4, space="PSUM"))
        ypool = ctx.enter_context(tc.tile_pool(name="yout", bufs=4))

        w1_sb = wpool.tile([P, N_D, H], bf16)   # 64 KB/partition
        w2_sb = wpool.tile([P, N_H, O], bf16)   # 64 KB/partition
        b1_sb = wpool.tile([P, N_H], f32)
        b2_sb = wpool.tile([P, N_O], f32)

        for hc in range(8):
            c0, c1 = hc * 512, (hc + 1) * 512
            for d in range(N_D):
                nc.sync.dma_start(out=w1_sb[:, d, c0:c1],
                                  in_=w1_d[d * P:(d + 1) * P, c0:c1])
            if hc == 0:
                nc.sync.dma_start(out=b1_sb[:], in_=b1_d[:])
                nc.sync.dma_start(out=b2_sb[:], in_=b2_d[:])
        for o in range(N_O):
            for h in range(N_H):
                nc.sync.dma_start(out=w2_sb[:, h, o * P:(o + 1) * P],
                                  in_=w2_d[h * P:(h + 1) * P,
                                           o * P:(o + 1) * P])

        gelu = mybir.ActivationFunctionType.Gelu

        loop_ctx = ExitStack()
        if iters > 1:
            loop_ctx.enter_context(tc.For_i(0, iters, 1))
        ctx.enter_context(loop_ctx)

        for grp in tile_groups:
            xts, hts = {}, {}
            for t in grp:
                t0, nt = token_tiles[t]
                xt_sb = xpool.tile([P, N_D, nt_max], bf16, tag="xt",
                                   name=f"xt{t}")
                for d in range(N_D):
                    nc.sync.dma_start(out=xt_sb[:, d, :nt],
                                      in_=xt_d[d * P:(d + 1) * P, t0:t0 + nt])
                xts[t] = xt_sb
                hts[t] = hpool.tile([P, N_H, nt_max], bf16, tag="ht",
                                    name=f"ht{t}")
            # Phase A over the group, matmuls per weight tile back-to-back
            for m in range(N_H):
                pss = {t: ppool1.tile([P, 512], f32, tag="ps1",
                                      name=f"psA{m}_{t}") for t in grp}
                for d in range(N_D):
                    lw = w1_sb[:, d, m * P:(m + 1) * P]
                    for t in grp:
                        nt = token_tiles[t][1]
                        nc.tensor.matmul(pss[t][:, :nt], lhsT=lw,
                                         rhs=xts[t][:, d, :nt],
                                         start=(d == 0), stop=(d == N_D - 1))
                for t in grp:
                    nt = token_tiles[t][1]
                    nc.scalar.activation(hts[t][:, m, :nt], pss[t][:, :nt],
                                         gelu, bias=b1_sb[:, m:m + 1])
            # Phase B over the group
            for o in range(N_O):
                pss = {t: ppool2.tile([P, 512], f32, tag="ps2",
                                      name=f"psB{o}_{t}") for t in grp}
                for h in range(N_H):
                    lw = w2_sb[:, h, o * P:(o + 1) * P]
                    for t in grp:
                        nt = token_tiles[t][1]
                        nc.tensor.matmul(pss[t][:, :nt], lhsT=lw,
                                         rhs=hts[t][:, h, :nt],
                                         start=(h == 0), stop=(h == N_H - 1))
                for t in grp:
                    t0, nt = token_tiles[t]
                    yb = ypool.tile([P, 512], bf16, tag="yb")
                    nc.vector.tensor_scalar_add(yb[:, :nt], pss[t][:, :nt],
                                                b2_sb[:, o:o + 1])
                    nc.sync.dma_start(out=yt_d[o * P:(o + 1) * P, t0:t0 + nt],
                                      in_=yb[:, :nt])
    removed = _dedup_ldweights(nc)
    nc.compile()
    return nc


MODE = os.environ.get("KMODE", "base")


def _prepare(x, Wg, W1, b1, W2, b2):
    """Host-side gating + per-expert gather. Returns (in_maps, glob, w, C, B, S)."""
    B, S, Dx = x.shape
    assert Dx == D and Wg.shape == (D, E), (x.shape, Wg.shape)
    T = B * S
    xf = np.ascontiguousarray(x.reshape(T, D), dtype=np.float32)
    logits = xf.astype(np.float64) @ Wg.astype(np.float64)
    top_i = np.argpartition(-logits, TOPK - 1, axis=1)[:, :TOPK]  # [T, 2]
    lv = np.take_along_axis(logits, top_i, axis=1)
    lv -= lv.max(axis=1, keepdims=True)
    ex = np.exp(lv)
    w = ex / ex.sum(axis=1, keepdims=True)  # [T, 2] softmax over the pair

    flat_e = top_i.reshape(-1)      # pair p = 2*t + k -> expert id
    flat_w = w.reshape(-1)
    counts = np.bincount(flat_e, minlength=E)
    # Tokens are the matmul free dim, so capacity needs no alignment at all;
    # every extra padded token costs PE time on all 8 cores.
    C = max(1024, int(counts.max()))

    xt_bf = np.ascontiguousarray(xf.T).astype(BF16)  # [D, T]
    W1b = W1.astype(BF16)
    W2b = W2.astype(BF16)

    in_maps = []
    glob = np.empty(2 * T, dtype=np.int64)  # pair -> row in stacked outputs
    for e in range(E):
        sel = np.nonzero(flat_e == e)[0]
        tok = sel >> 1
        n = len(sel)
        xt_e = np.zeros((D, C), dtype=BF16)
        xt_e[:, :n] = xt_bf[:, tok]
        glob[sel] = e * C + np.arange(n)
        in_maps.append({
            "xt": xt_e,
            "w1": np.ascontiguousarray(W1b[e]),
            "w2": np.ascontiguousarray(W2b[e]),
            # [128, m] with b[p, m] = bias[m*128 + p]
            "b1": np.ascontiguousarray(
                np.asarray(b1[e], dtype=np.float32).reshape(N_H, P).T),
            "b2": np.ascontiguousarray(
                np.asarray(b2[e], dtype=np.float32).reshape(N_O, P).T),
        })
    return in_maps, glob, flat_w, C, B, S


def _get_nc(C: int, iters: int = 1) -> bass.Bass:
    key = (MODE, C, iters)
    nc = _CACHE.get(key)
    if nc is None:
        build = {"ilv": _build_ilv, "ilv2": _build_ilv2}.get(MODE, _build)
        nc = _CACHE[key] = build(C, iters)
    return nc


def _combine(results, glob, flat_w, C, B, S):
    # yt arrives bf16 [O, C]; gate weights are applied here (host), so the
    # device never needs them and padding rows are simply never gathered.
    Y = np.stack([np.asarray(r["yt"]).astype(np.float32).T for r in results])
    Yflat = Y.reshape(E * C, O)
    out = (flat_w[0::2, None].astype(np.float32) * Yflat[glob[0::2]]
           + flat_w[1::2, None].astype(np.float32) * Yflat[glob[1::2]])
    return out.reshape(B, S, O).astype(np.float32, copy=False)


def kernel(x, Wg, W1, b1, W2, b2):
    in_maps, glob, flat_w, C, B, S = _prepare(x, Wg, W1, b1, W2, b2)
    nc = _get_nc(C)
    res = run_bass_kernel_spmd(nc, in_maps, core_ids=list(range(N_CORES)))
    return _combine(res.results, glob, flat_w, C, B, S)

